# revision 7
# baseline (speedup 1.0000x reference)
"""Trainium2 Bass kernel for nn_MoIETransformerBlock (8-core SPMD).

Sharding: core c -> batch b=c//2, half h=c%2. Each core owns 8 query
tiles of 128 rows (half0: global tiles [15,14,13,12,3,2,1,0], half1:
[11,10,9,8,7,6,5,4]); both halves share one padded scores-length
profile LT so the SPMD program is uniform; causality and padding are a
host-built additive mask. proto_state is computed in 8 row-slices and
AllGathered; kT/v are computed for own rows and pair-exchanged.
All matmuls run in true fp32 (the (mv-cost)>0 hard gates need it).
"""
import numpy as np

import concourse.bass as bass
import concourse.mybir as mybir
import concourse.tile as tile
from concourse import bacc

F32 = mybir.dt.float32
AX = mybir.AxisListType
ALU = mybir.AluOpType
ACTF = mybir.ActivationFunctionType

B, S, D = 4, 2048, 1024
NT = D // 128          # 8 d-tiles
ST = 8                 # 8 local s-tiles (1024 own rows)
SLOC = ST * 128
LN_EPS = 1e-5
GATE_EPS = 1e-9
SCALE = 1.0 / 32.0
NEG = -1e30

TILE_ORDER = {0: [15, 14, 13, 12, 3, 2, 1, 0], 1: [11, 10, 9, 8, 7, 6, 5, 4]}
LT = [2048, 1920, 1792, 1664, 1024, 896, 768, 640]
MAXCHUNK = 4

KEYSRC = {}
for _h in (0, 1):
    for _l, _tau in enumerate(TILE_ORDER[_h]):
        KEYSRC[_tau] = (_h, _l)


def build():
    from contextlib import ExitStack

    nc = bacc.Bacc(num_devices=8)

    x_own = nc.dram_tensor("x_own", [SLOC, D], F32, kind="ExternalInput")
    ln1_g = nc.dram_tensor("ln1_g", [1, D], F32, kind="ExternalInput")
    ln1_b = nc.dram_tensor("ln1_b", [1, D], F32, kind="ExternalInput")
    mu_wT = nc.dram_tensor("mu_wT", [4, D, D], F32, kind="ExternalInput")   # [n,w,u]
    mu_b = nc.dram_tensor("mu_b", [4, D], F32, kind="ExternalInput")
    gate = nc.dram_tensor("gate", [4, D], F32, kind="ExternalInput")
    pt_wT = nc.dram_tensor("pt_wT", [4, D, D], F32, kind="ExternalInput")   # [n,t,w]
    pln_g = nc.dram_tensor("pln_g", [4, D], F32, kind="ExternalInput")
    pln_b = nc.dram_tensor("pln_b", [4, D], F32, kind="ExternalInput")
    ip_Ts = nc.dram_tensor("ip_Ts", [4, D, 128], F32, kind="ExternalInput")
    pw_s = nc.dram_tensor("pw_s", [4, 128, D], F32, kind="ExternalInput")
    amask = nc.dram_tensor("amask", [ST, 128, S], F32, kind="ExternalInput")
    ident_in = nc.dram_tensor("ident_in", [128, 128], F32, kind="ExternalInput")

    y = nc.dram_tensor("y", [SLOC, D], F32, kind="ExternalOutput")

    proto_in = [nc.dram_tensor(f"proto_in{n}", [D, 128], F32) for n in range(4)]
    proto_all = [
        nc.dram_tensor(f"proto_all{n}", [8 * D, 128], F32, addr_space="Shared")
        for n in range(4)
    ]
    kv_in = nc.dram_tensor("kv_in", [2 * SLOC, D], F32)
    kv_all = nc.dram_tensor("kv_all", [2 * 2 * SLOC, D], F32)

    def protoT_tile(j, n, i):
        """proto_stateT stationary tile [w-tile i, u-block j] as DRAM AP."""
        r0 = j * D + i * 128
        return proto_all[n][r0:r0 + 128, :]

    def bcast_ap(row_ap):
        """[1, n] DRAM AP -> [128, n] partition-broadcast AP."""
        return bass.AP(tensor=row_ap.tensor, offset=row_ap.offset,
                       ap=[[0, 128]] + row_ap.ap[1:])

    with tile.TileContext(nc) as tc, ExitStack() as ex:
        con = ex.enter_context(tc.tile_pool(name="con", bufs=1))
        wrk = ex.enter_context(tc.tile_pool(name="wrk", bufs=2))
        ps = ex.enter_context(tc.tile_pool(name="ps", bufs=2, space="PSUM"))
        pst = ex.enter_context(tc.tile_pool(name="pst", bufs=2, space="PSUM"))

        # ---------- constants ----------
        ident = con.tile([128, 128], F32)
        nc.sync.dma_start(out=ident, in_=ident_in[:, :])
        ones_row = con.tile([1, 128], F32)
        nc.vector.memset(ones_row, 1.0)
        eps_t = con.tile([128, 1], F32)
        nc.vector.memset(eps_t, LN_EPS)

        mub_t = con.tile([128, 2, NT], F32)
        nc.sync.dma_start(
            out=mub_t,
            in_=bass.AP(tensor=mu_b, offset=0,
                        ap=[[1, 128], [D, 2], [128, NT]]))
        gate_t = con.tile([128, 4, NT], F32)
        nc.sync.dma_start(
            out=gate_t,
            in_=bass.AP(tensor=gate, offset=0,
                        ap=[[1, 128], [D, 4], [128, NT]]))

        m1 = wrk.tile([128, 4], F32, tag="gm")
        nc.vector.tensor_reduce(m1, gate_t, AX.X, ALU.max, apply_absolute_value=True)
        m2 = wrk.tile([1, 4], F32, tag="gm2")
        nc.gpsimd.tensor_reduce(m2, m1, AX.C, ALU.max, apply_absolute_value=True)
        nc.vector.tensor_scalar_add(m2, m2, GATE_EPS)
        inv2 = wrk.tile([1, 4], F32, tag="gm3")
        nc.vector.reciprocal(inv2, m2)
        pinv = pst.tile([128, 4], F32, tag="tp")
        nc.tensor.matmul(pinv, ones_row, inv2, start=True, stop=True)
        invb = con.tile([128, 4], F32)
        nc.scalar.copy(invb, pinv)

        cost32_t = con.tile([128, 2, NT], F32)
        for n in range(2):
            nc.vector.tensor_scalar(
                out=cost32_t[:, n, :], in0=gate_t[:, n, :],
                scalar1=invb[:, n:n + 1], scalar2=32.0, op0=ALU.mult, op1=ALU.mult)
        cost32b = {}
        mub_b = {}
        for n in (2, 3):
            gb = wrk.tile([128, D], F32, tag="gb")
            nc.sync.dma_start(out=gb, in_=bcast_ap(gate[n:n + 1, :]))
            cb = con.tile([128, D], F32, tag=f"cost32b{n}")
            nc.vector.tensor_scalar(out=cb, in0=gb, scalar1=invb[:, n:n + 1],
                                    scalar2=32.0, op0=ALU.mult, op1=ALU.mult)
            cost32b[n] = cb
            mb = con.tile([128, D], F32, tag=f"mub_b{n}")
            nc.sync.dma_start(out=mb, in_=bcast_ap(mu_b[n:n + 1, :]))
            mub_b[n] = mb

        def layernorm_rows(pool, dst, src_t, gt, bt):
            sub = src_t.rearrange("p (a q) -> p a q", a=2)
            stats = pool.tile([128, 2, 6], F32, tag="lnst")
            for a in range(2):
                nc.vector.bn_stats(out=stats[:, a, :], in_=sub[:, a, :])
            mv = pool.tile([128, 2], F32, tag="lnmv")
            nc.vector.bn_aggr(out=mv, in_=stats)
            rstd = pool.tile([128, 1], F32, tag="lnr")
            nc.scalar.activation(out=rstd, in_=mv[:, 1:2], func=ACTF.Sqrt,
                                 bias=eps_t, scale=1.0)
            nc.vector.reciprocal(rstd, rstd)
            nc.vector.tensor_scalar(out=dst, in0=src_t, scalar1=mv[:, 0:1],
                                    scalar2=rstd, op0=ALU.subtract, op1=ALU.mult)
            nc.vector.tensor_tensor(out=dst, in0=dst, in1=gt, op=ALU.mult)
            nc.vector.tensor_tensor(out=dst, in0=dst, in1=bt, op=ALU.add)

        def transpose128(dst, src):
            pt = pst.tile([128, 128], F32, tag="tp")
            nc.tensor.transpose(pt, src, ident)
            nc.scalar.copy(dst, pt)

        # ---------- proto_state slices + AllGather (per n) ----------
        with tc.tile_pool(name="proto", bufs=1) as pp:
            for n in range(4):
                plg = pp.tile([128, D], F32, tag="plg")
                nc.sync.dma_start(out=plg, in_=bcast_ap(pln_g[n:n + 1, :]))
                plb = pp.tile([128, D], F32, tag="plb")
                nc.sync.dma_start(out=plb, in_=bcast_ap(pln_b[n:n + 1, :]))
                ipT = pp.tile([128, NT, 128], F32, tag="ipT")
                nc.sync.dma_start(
                    out=ipT, in_=ip_Ts[n].rearrange("(i p) u -> p i u", p=128))
                ptT = pp.tile([128, NT, D], F32, tag="ptT")
                nc.sync.dma_start(
                    out=ptT, in_=pt_wT[n].rearrange("(i p) w -> p i w", p=128))
                prc = pp.tile([128, D], F32, tag="prc")
                for c in range(2):
                    pm = ps.tile([128, 512], F32, tag="mm")
                    for i in range(NT):
                        nc.tensor.matmul(pm, ipT[:, i, :],
                                         ptT[:, i, c * 512:(c + 1) * 512],
                                         start=(i == 0), stop=(i == NT - 1))
                    nc.scalar.copy(prc[:, c * 512:(c + 1) * 512], pm)
                psl = pp.tile([128, D], F32, tag="psl")
                layernorm_rows(pp, psl, prc, plg, plb)
                pw = pp.tile([128, D], F32, tag="pw")
                nc.sync.dma_start(out=pw, in_=pw_s[n])
                nc.vector.tensor_tensor(out=psl, in0=psl, in1=pw, op=ALU.add)
                psT = pp.tile([128, NT, 128], F32, tag="psT")
                for i in range(NT):
                    transpose128(psT[:, i, :], psl[:, i * 128:(i + 1) * 128])
                nc.sync.dma_start(
                    out=proto_in[n][:, :].rearrange("(i p) u -> p i u", p=128),
                    in_=psT)
                nc.gpsimd.collective_compute(
                    "AllGather", ALU.bypass, replica_groups=[list(range(8))],
                    ins=[proto_in[n][:, :].opt()], outs=[proto_all[n][:, :].opt()])

        # ---------- h = LN(x) -> hT; q,k,v; kv exchange ----------
        hT_stack = ExitStack()
        pool_hT = hT_stack.enter_context(tc.tile_pool(name="hTp", bufs=1))
        hT = [pool_hT.tile([128, SLOC], F32, tag=f"hT{i}", name=f"hT{i}") for i in range(NT)]
        with tc.tile_pool(name="hwrk", bufs=2) as hw:
            gb_ = hw.tile([128, D], F32, tag="g")
            nc.sync.dma_start(out=gb_, in_=bcast_ap(ln1_g[0:1, :]))
            bb_ = hw.tile([128, D], F32, tag="b")
            nc.sync.dma_start(out=bb_, in_=bcast_ap(ln1_b[0:1, :]))
            for st in range(ST):
                xt = hw.tile([128, D], F32, tag="xt")
                nc.sync.dma_start(out=xt, in_=x_own[st * 128:(st + 1) * 128, :])
                ht = hw.tile([128, D], F32, tag="ht")
                layernorm_rows(hw, ht, xt, gb_, bb_)
                for i in range(NT):
                    transpose128(hT[i][:, st * 128:(st + 1) * 128],
                                 ht[:, i * 128:(i + 1) * 128])

        qT_stack = ExitStack()
        pool_qT = qT_stack.enter_context(tc.tile_pool(name="qTp", bufs=1, side="right"))
        qT = [pool_qT.tile([128, SLOC], F32, tag=f"qT{i}", name=f"qT{i}") for i in range(NT)]

        # q, k in T-layout
        with tc.tile_pool(name="statqk", bufs=2) as sq, \
             tc.tile_pool(name="ktp", bufs=2) as kp:
            for n in range(2):
                for j in range(NT):
                    mus = sq.tile([128, NT, 128], F32, tag="mus")
                    nc.sync.dma_start(
                        out=mus,
                        in_=mu_wT[n, :, j * 128:(j + 1) * 128].rearrange(
                            "(i p) u -> p i u", p=128))
                    prs = sq.tile([128, NT, 128], F32, tag="prs")
                    for i in range(NT):
                        nc.sync.dma_start(out=prs[:, i, :], in_=protoT_tile(j, n, i))
                    dst = (qT[j] if n == 0 else
                           kp.tile([128, SLOC], F32, tag="kT", name="kTj"))
                    for c in range(2):
                        sl = slice(c * 512, (c + 1) * 512)
                        pc = ps.tile([128, 512], F32, tag="mm")
                        for i in range(NT):
                            nc.tensor.matmul(pc, mus[:, i, :], hT[i][:, sl],
                                             start=(i == 0), stop=(i == NT - 1))
                        pm = ps.tile([128, 512], F32, tag="mm2")
                        for i in range(NT):
                            nc.tensor.matmul(pm, prs[:, i, :], hT[i][:, sl],
                                             start=(i == 0), stop=(i == NT - 1))
                        comp = wrk.tile([128, 512], F32, tag="comp")
                        nc.scalar.activation(out=comp, in_=pc, func=ACTF.Silu,
                                             bias=mub_t[:, n, j:j + 1], scale=1.0)
                        msk = wrk.tile([128, 512], F32, tag="msk")
                        nc.vector.tensor_scalar(
                            out=msk, in0=pm, scalar1=cost32_t[:, n, j:j + 1],
                            scalar2=0.0, op0=ALU.subtract, op1=ALU.is_gt)
                        nc.vector.tensor_tensor(out=dst[:, sl], in0=comp, in1=msk,
                                                op=ALU.mult)
                    if n == 1:
                        nc.sync.dma_start(
                            out=kv_in[j * 128:(j + 1) * 128, :], in_=dst)

        # v in N-layout, written straight to kv_in
        with tc.tile_pool(name="statv", bufs=1) as sv:
            for uc in range(4):
                usl = slice(uc * 256, (uc + 1) * 256)
                muv = sv.tile([128, NT, 256], F32, tag="muv")
                nc.sync.dma_start(
                    out=muv,
                    in_=mu_wT[2, :, usl].rearrange("(i p) u -> p i u", p=128))
                prv = sv.tile([128, NT, 256], F32, tag="prv")
                for i in range(NT):
                    for jj in range(2):
                        j = uc * 2 + jj
                        nc.sync.dma_start(out=prv[:, i, jj * 128:(jj + 1) * 128],
                                          in_=protoT_tile(j, 2, i))
                for st in range(ST):
                    ssl = slice(st * 128, (st + 1) * 128)
                    pc = ps.tile([128, 512], F32, tag="mm")
                    for i in range(NT):
                        nc.tensor.matmul(pc[:, 0:256], hT[i][:, ssl], muv[:, i, :],
                                         start=(i == 0), stop=(i == NT - 1))
                    pm = ps.tile([128, 512], F32, tag="mm2")
                    for i in range(NT):
                        nc.tensor.matmul(pm[:, 0:256], hT[i][:, ssl], prv[:, i, :],
                                         start=(i == 0), stop=(i == NT - 1))
                    comp = wrk.tile([128, 512], F32, tag="comp")
                    nc.vector.tensor_tensor(out=comp[:, 0:256], in0=pc[:, 0:256],
                                            in1=mub_b[2][:, usl], op=ALU.add)
                    nc.scalar.activation(out=comp[:, 0:256], in_=comp[:, 0:256],
                                         func=ACTF.Silu)
                    msk = wrk.tile([128, 512], F32, tag="msk")
                    nc.vector.tensor_tensor(out=msk[:, 0:256], in0=pm[:, 0:256],
                                            in1=cost32b[2][:, usl], op=ALU.subtract)
                    nc.vector.tensor_scalar(out=msk[:, 0:256], in0=msk[:, 0:256],
                                            scalar1=0.0, scalar2=None, op0=ALU.is_gt)
                    vout = wrk.tile([128, 256], F32, tag="vout")
                    nc.vector.tensor_tensor(out=vout, in0=comp[:, 0:256],
                                            in1=msk[:, 0:256], op=ALU.mult)
                    nc.sync.dma_start(
                        out=kv_in[SLOC + st * 128:SLOC + (st + 1) * 128, usl],
                        in_=vout)
        hT_stack.close()

        nc.gpsimd.collective_compute(
            "AllGather", ALU.bypass,
            replica_groups=[[0, 1], [2, 3], [4, 5], [6, 7]],
            ins=[kv_in[:, :].opt()], outs=[kv_all[:, :].opt()])

        # ---------- scores + softmax ----------
        P_stack = ExitStack()
        pool_P = P_stack.enter_context(tc.tile_pool(name="Pp", bufs=1))
        P = [pool_P.tile([128, LT[t]], F32, tag=f"P{t}", name=f"P{t}") for t in range(ST)]
        with tc.tile_pool(name="statk", bufs=2) as sk:
            for c in range(MAXCHUNK):
                kch = sk.tile([128, NT, 512], F32, tag="kch")
                for i in range(NT):
                    for kb in range(4):
                        tau = c * 4 + kb
                        p_, l_ = KEYSRC[tau]
                        r0 = p_ * 2 * SLOC + i * 128
                        nc.sync.dma_start(
                            out=kch[:, i, kb * 128:(kb + 1) * 128],
                            in_=kv_all[r0:r0 + 128, l_ * 128:(l_ + 1) * 128])
                for t in range(ST):
                    if LT[t] <= c * 512:
                        continue
                    n_c = min(512, LT[t] - c * 512)
                    pp_ = ps.tile([128, 512], F32, tag="mm")
                    for i in range(NT):
                        nc.tensor.matmul(pp_[:, :n_c],
                                         qT[i][:, t * 128:(t + 1) * 128],
                                         kch[:, i, :n_c],
                                         start=(i == 0), stop=(i == NT - 1))
                    mt = wrk.tile([128, 512], F32, tag="amsk")
                    nc.sync.dma_start(out=mt[:, :n_c],
                                      in_=amask[t, :, c * 512:c * 512 + n_c])
                    seg = P[t][:, c * 512:c * 512 + n_c]
                    nc.scalar.activation(out=seg, in_=pp_[:, :n_c], func=ACTF.Copy,
                                         scale=SCALE)
                    nc.vector.tensor_tensor(out=seg, in0=seg, in1=mt[:, :n_c],
                                            op=ALU.add)
        qT_stack.close()

        at_stack = ExitStack()
        pool_at = at_stack.enter_context(tc.tile_pool(name="atp", bufs=1, side="right"))
        rinv = pool_at.tile([128, ST], F32, tag="rinv")
        for t in range(ST):
            nmax = wrk.tile([128, 1], F32, tag="nmax")
            nc.vector.tensor_reduce(nmax, P[t], AX.X, ALU.max, negate=True)
            nc.scalar.activation(out=P[t], in_=P[t], func=ACTF.Exp, bias=nmax,
                                 scale=1.0)
            rs = wrk.tile([128, 1], F32, tag="rs")
            nc.vector.tensor_reduce(rs, P[t], AX.X, ALU.add)
            nc.vector.reciprocal(rinv[:, t:t + 1], rs)

        # ---------- attn = P @ v ----------
        attn = [pool_at.tile([128, D], F32, tag=f"at{t}", name=f"at{t}") for t in range(ST)]
        with tc.tile_pool(name="statvch", bufs=2) as sva:
            for c in range(MAXCHUNK):
                vch = sva.tile([128, 4, D], F32, tag="vch")
                for kb in range(4):
                    tau = c * 4 + kb
                    p_, l_ = KEYSRC[tau]
                    r0 = p_ * 2 * SLOC + SLOC + l_ * 128
                    nc.sync.dma_start(out=vch[:, kb, :], in_=kv_all[r0:r0 + 128, :])
                for t in range(ST):
                    if LT[t] <= c * 512:
                        continue
                    n_c = min(512, LT[t] - c * 512)
                    nkb = n_c // 128
                    PT = wrk.tile([128, 4, 128], F32, tag="PT")
                    for kb in range(nkb):
                        transpose128(
                            PT[:, kb, :],
                            P[t][:, c * 512 + kb * 128:c * 512 + (kb + 1) * 128])
                    for fc in range(2):
                        fsl = slice(fc * 512, (fc + 1) * 512)
                        pa = ps.tile([128, 512], F32, tag="mm")
                        for kb in range(nkb):
                            nc.tensor.matmul(pa, PT[:, kb, :], vch[:, kb, fsl],
                                             start=(kb == 0), stop=(kb == nkb - 1))
                        if c == 0:
                            nc.scalar.copy(attn[t][:, fsl], pa)
                        else:
                            nc.vector.tensor_tensor(out=attn[t][:, fsl],
                                                    in0=attn[t][:, fsl], in1=pa,
                                                    op=ALU.add)
        P_stack.close()
        for t in range(ST):
            nc.vector.tensor_scalar_mul(attn[t], attn[t], rinv[:, t:t + 1])

        # ---------- attn_outT; out-proj + residual ----------
        aT_stack = ExitStack()
        pool_aT = aT_stack.enter_context(tc.tile_pool(name="aTp", bufs=1))
        aT = [pool_aT.tile([128, SLOC], F32, tag=f"aT{i}", name=f"aT{i}") for i in range(NT)]
        for t in range(ST):
            for i in range(NT):
                transpose128(aT[i][:, t * 128:(t + 1) * 128],
                             attn[t][:, i * 128:(i + 1) * 128])
        at_stack.close()

        with tc.tile_pool(name="stato", bufs=1) as so:
            for uc in range(4):
                usl = slice(uc * 256, (uc + 1) * 256)
                muo = so.tile([128, NT, 256], F32, tag="muo")
                nc.sync.dma_start(
                    out=muo,
                    in_=mu_wT[3, :, usl].rearrange("(i p) u -> p i u", p=128))
                pro = so.tile([128, NT, 256], F32, tag="pro")
                for i in range(NT):
                    for jj in range(2):
                        j = uc * 2 + jj
                        nc.sync.dma_start(out=pro[:, i, jj * 128:(jj + 1) * 128],
                                          in_=protoT_tile(j, 3, i))
                for st in range(ST):
                    ssl = slice(st * 128, (st + 1) * 128)
                    pc = ps.tile([128, 512], F32, tag="mm")
                    for i in range(NT):
                        nc.tensor.matmul(pc[:, 0:256], aT[i][:, ssl], muo[:, i, :],
                                         start=(i == 0), stop=(i == NT - 1))
                    pm = ps.tile([128, 512], F32, tag="mm2")
                    for i in range(NT):
                        nc.tensor.matmul(pm[:, 0:256], aT[i][:, ssl], pro[:, i, :],
                                         start=(i == 0), stop=(i == NT - 1))
                    comp = wrk.tile([128, 512], F32, tag="comp")
                    nc.vector.tensor_tensor(out=comp[:, 0:256], in0=pc[:, 0:256],
                                            in1=mub_b[3][:, usl], op=ALU.add)
                    nc.scalar.activation(out=comp[:, 0:256], in_=comp[:, 0:256],
                                         func=ACTF.Silu)
                    msk = wrk.tile([128, 512], F32, tag="msk")
                    nc.vector.tensor_tensor(out=msk[:, 0:256], in0=pm[:, 0:256],
                                            in1=cost32b[3][:, usl], op=ALU.subtract)
                    nc.vector.tensor_scalar(out=msk[:, 0:256], in0=msk[:, 0:256],
                                            scalar1=0.0, scalar2=None, op0=ALU.is_gt)
                    nc.vector.tensor_tensor(out=msk[:, 0:256], in0=comp[:, 0:256],
                                            in1=msk[:, 0:256], op=ALU.mult)
                    xt = wrk.tile([128, 256], F32, tag="xres")
                    nc.sync.dma_start(out=xt, in_=x_own[ssl, usl])
                    nc.vector.tensor_tensor(out=msk[:, 0:256], in0=msk[:, 0:256],
                                            in1=xt, op=ALU.add)
                    nc.sync.dma_start(out=y[ssl, usl], in_=msk[:, 0:256])
        aT_stack.close()

    nc.compile()
    return nc


# ======================= host side =======================

_RUNNER = None


class _SpmdRunner:
    def __init__(self, nc, n_cores):
        import jax
        from jax.sharding import Mesh, PartitionSpec
        from jax.experimental.shard_map import shard_map
        from concourse.bass2jax import (
            _bass_exec_p, partition_id_tensor, install_neuronx_cc_hook)

        install_neuronx_cc_hook()
        self.jax = jax
        self.n_cores = n_cores
        partition_name = nc.partition_id_tensor.name if nc.partition_id_tensor else None
        in_names, out_names, out_avals = [], [], []
        for alloc in nc.m.functions[0].allocations:
            if not isinstance(alloc, mybir.MemoryLocationSet):
                continue
            name = alloc.memorylocations[0].name
            if alloc.kind == "ExternalInput":
                if name != partition_name:
                    in_names.append(name)
            elif alloc.kind == "ExternalOutput":
                out_names.append(name)
                out_avals.append(jax.core.ShapedArray(
                    tuple(alloc.tensor_shape), mybir.dt.np(alloc.dtype)))
        self.in_names, self.out_names, self.out_avals = in_names, out_names, out_avals
        n_params, n_outs = len(in_names), len(out_avals)

        def _body(*args):
            operands = list(args)
            if partition_name is not None:
                operands.append(partition_id_tensor())
            outs = _bass_exec_p.bind(
                *operands,
                out_avals=tuple(out_avals),
                in_names=tuple(in_names + out_names
                               + ([partition_name] if partition_name else [])),
                out_names=tuple(out_names),
                lowering_input_output_aliases=(),
                sim_require_finite=False,
                sim_require_nnan=False,
                nc=nc,
            )
            return tuple(outs)

        devices = jax.devices()[:n_cores]
        mesh = Mesh(np.asarray(devices), ("core",))
        in_specs = (PartitionSpec("core"),) * (n_params + n_outs)
        out_specs = (PartitionSpec("core"),) * n_outs
        self.fn = jax.jit(
            shard_map(_body, mesh=mesh, in_specs=in_specs, out_specs=out_specs,
                      check_rep=False),
            keep_unused=True)

    def run(self, in_maps):
        concat_in = [
            np.ascontiguousarray(
                np.concatenate([np.asarray(m[name]) for m in in_maps], axis=0))
            for name in self.in_names
        ]
        concat_zeros = [
            np.zeros((self.n_cores * a.shape[0], *a.shape[1:]), a.dtype)
            for a in self.out_avals
        ]
        outs = self.fn(*concat_in, *concat_zeros)
        self.jax.block_until_ready(outs)
        res = []
        for c in range(self.n_cores):
            d = {}
            for i, name in enumerate(self.out_names):
                full = np.asarray(outs[i])
                d[name] = full.reshape(self.n_cores, *self.out_avals[i].shape)[c]
            res.append(d)
        return res


def _build_mask(h):
    m = np.full((ST, 128, S), NEG, np.float32)
    order = TILE_ORDER[h]
    keys = np.arange(S)
    for t, tau in enumerate(order):
        rows = tau * 128 + np.arange(128)
        m[t] = np.where(keys[None, :] <= rows[:, None], 0.0, NEG)
        m[t, :, LT[t]:] = NEG
    return m


def make_in_maps(x, ln1_g, ln1_b, mu_w, mu_b, proto_w, gate, pt_w, pln_g,
                 pln_b, in_proto):
    mu_wT = np.ascontiguousarray(np.transpose(mu_w, (0, 2, 1)))
    pt_wT = np.ascontiguousarray(np.transpose(pt_w, (0, 2, 1)))
    ident = np.eye(128, dtype=np.float32)
    masks = {h: _build_mask(h) for h in (0, 1)}
    in_maps = []
    for c in range(8):
        b, h = c // 2, c % 2
        order = TILE_ORDER[h]
        x_own = np.concatenate(
            [x[b, tau * 128:(tau + 1) * 128, :] for tau in order], axis=0)
        ip_Ts = np.ascontiguousarray(
            np.transpose(in_proto[:, c * 128:(c + 1) * 128, :], (0, 2, 1)))
        pw_s = np.ascontiguousarray(proto_w[:, c * 128:(c + 1) * 128, :])
        in_maps.append({
            "x_own": np.ascontiguousarray(x_own),
            "ln1_g": np.asarray(ln1_g).reshape(1, D),
            "ln1_b": np.asarray(ln1_b).reshape(1, D),
            "mu_wT": mu_wT, "mu_b": mu_b, "gate": gate,
            "pt_wT": pt_wT, "pln_g": pln_g, "pln_b": pln_b,
            "ip_Ts": ip_Ts, "pw_s": pw_s,
            "amask": masks[h], "ident_in": ident,
        })
    return in_maps


def kernel(**inputs):
    global _RUNNER
    inputs = {k: np.asarray(v, np.float32) for k, v in inputs.items()}
    in_maps = make_in_maps(**inputs)
    if _RUNNER is None:
        _RUNNER = _SpmdRunner(build(), 8)
    res = _RUNNER.run(in_maps)
    out = np.empty((B, S, D), np.float32)
    for c in range(8):
        b, h = c // 2, c % 2
        yc = res[c]["y"]
        for t, tau in enumerate(TILE_ORDER[h]):
            out[b, tau * 128:(tau + 1) * 128, :] = yc[t * 128:(t + 1) * 128, :]
    return out


# revision 8
# speedup vs baseline: 1.0020x; 1.0020x over previous
"""Trainium2 Bass kernel for nn_MoIETransformerBlock (8-core SPMD).

Sharding: core c -> batch b=c//2, half h=c%2. Each core owns 8 query
tiles of 128 rows (half0: global tiles [15,14,13,12,3,2,1,0], half1:
[11,10,9,8,7,6,5,4]); both halves share one padded scores-length
profile LT so the SPMD program is uniform; causality and padding are a
host-built additive mask. proto_state is computed in 8 row-slices and
AllGathered; kT/v are computed for own rows and pair-exchanged.
All matmuls run in true fp32 (the (mv-cost)>0 hard gates need it).
"""
import numpy as np

import concourse.bass as bass
import concourse.mybir as mybir
import concourse.tile as tile
from concourse import bacc

F32 = mybir.dt.float32
AX = mybir.AxisListType
ALU = mybir.AluOpType
ACTF = mybir.ActivationFunctionType

B, S, D = 4, 2048, 1024
NT = D // 128          # 8 d-tiles
ST = 8                 # 8 local s-tiles (1024 own rows)
SLOC = ST * 128
LN_EPS = 1e-5
GATE_EPS = 1e-9
SCALE = 1.0 / 32.0
NEG = -1e30

TILE_ORDER = {0: [15, 14, 13, 12, 3, 2, 1, 0], 1: [11, 10, 9, 8, 7, 6, 5, 4]}
LT = [2048, 1920, 1792, 1664, 1024, 896, 768, 640]
MAXCHUNK = 4

KEYSRC = {}
for _h in (0, 1):
    for _l, _tau in enumerate(TILE_ORDER[_h]):
        KEYSRC[_tau] = (_h, _l)


def build(use_cc=True):
    from contextlib import ExitStack

    nc = bacc.Bacc(num_devices=8)

    x_own = nc.dram_tensor("x_own", [SLOC, D], F32, kind="ExternalInput")
    ln1_g = nc.dram_tensor("ln1_g", [1, D], F32, kind="ExternalInput")
    ln1_b = nc.dram_tensor("ln1_b", [1, D], F32, kind="ExternalInput")
    mu_wT = nc.dram_tensor("mu_wT", [4, D, D], F32, kind="ExternalInput")   # [n,w,u]
    mu_b = nc.dram_tensor("mu_b", [4, D], F32, kind="ExternalInput")
    gate = nc.dram_tensor("gate", [4, D], F32, kind="ExternalInput")
    pt_wT = nc.dram_tensor("pt_wT", [4, D, D], F32, kind="ExternalInput")   # [n,t,w]
    pln_g = nc.dram_tensor("pln_g", [4, D], F32, kind="ExternalInput")
    pln_b = nc.dram_tensor("pln_b", [4, D], F32, kind="ExternalInput")
    ip_Ts = nc.dram_tensor("ip_Ts", [4, D, 128], F32, kind="ExternalInput")
    pw_s = nc.dram_tensor("pw_s", [4, 128, D], F32, kind="ExternalInput")
    amask = nc.dram_tensor("amask", [ST, 128, S], F32, kind="ExternalInput")
    ident_in = nc.dram_tensor("ident_in", [128, 128], F32, kind="ExternalInput")

    y = nc.dram_tensor("y", [SLOC, D], F32, kind="ExternalOutput")

    proto_in = [nc.dram_tensor(f"proto_in{n}", [D, 128], F32) for n in range(4)]
    proto_all = [
        nc.dram_tensor(f"proto_all{n}", [8 * D, 128], F32, addr_space="Shared")
        for n in range(4)
    ]
    kv_in = nc.dram_tensor("kv_in", [2 * SLOC, D], F32)
    kv_all = nc.dram_tensor("kv_all", [2 * 2 * SLOC, D], F32)

    def protoT_tile(j, n, i):
        """proto_stateT stationary tile [w-tile i, u-block j] as DRAM AP."""
        r0 = j * D + i * 128
        return proto_all[n][r0:r0 + 128, :]

    def bcast_ap(row_ap):
        """[1, n] DRAM AP -> [128, n] partition-broadcast AP."""
        return bass.AP(tensor=row_ap.tensor, offset=row_ap.offset,
                       ap=[[0, 128]] + row_ap.ap[1:])

    with tile.TileContext(nc) as tc, ExitStack() as ex:
        con = ex.enter_context(tc.tile_pool(name="con", bufs=1))
        wrk = ex.enter_context(tc.tile_pool(name="wrk", bufs=2))
        ps = ex.enter_context(tc.tile_pool(name="ps", bufs=2, space="PSUM"))
        pst = ex.enter_context(tc.tile_pool(name="pst", bufs=2, space="PSUM"))

        # ---------- constants ----------
        ident = con.tile([128, 128], F32)
        nc.sync.dma_start(out=ident, in_=ident_in[:, :])
        ones_row = con.tile([1, 128], F32)
        nc.vector.memset(ones_row, 1.0)
        eps_t = con.tile([128, 1], F32)
        nc.vector.memset(eps_t, LN_EPS)

        mub_t = con.tile([128, 2, NT], F32)
        nc.sync.dma_start(
            out=mub_t,
            in_=bass.AP(tensor=mu_b, offset=0,
                        ap=[[1, 128], [D, 2], [128, NT]]))
        gate_t = con.tile([128, 4, NT], F32)
        nc.sync.dma_start(
            out=gate_t,
            in_=bass.AP(tensor=gate, offset=0,
                        ap=[[1, 128], [D, 4], [128, NT]]))

        m1 = wrk.tile([128, 4], F32, tag="gm")
        nc.vector.tensor_reduce(m1, gate_t, AX.X, ALU.max, apply_absolute_value=True)
        m2 = wrk.tile([1, 4], F32, tag="gm2")
        nc.gpsimd.tensor_reduce(m2, m1, AX.C, ALU.max, apply_absolute_value=True)
        nc.vector.tensor_scalar_add(m2, m2, GATE_EPS)
        inv2 = wrk.tile([1, 4], F32, tag="gm3")
        nc.vector.reciprocal(inv2, m2)
        pinv = pst.tile([128, 4], F32, tag="tp")
        nc.tensor.matmul(pinv, ones_row, inv2, start=True, stop=True)
        invb = con.tile([128, 4], F32)
        nc.scalar.copy(invb, pinv)

        cost32_t = con.tile([128, 2, NT], F32)
        for n in range(2):
            nc.vector.tensor_scalar(
                out=cost32_t[:, n, :], in0=gate_t[:, n, :],
                scalar1=invb[:, n:n + 1], scalar2=32.0, op0=ALU.mult, op1=ALU.mult)
        cost32b = {}
        mub_b = {}
        for n in (2, 3):
            gb = wrk.tile([128, D], F32, tag="gb")
            nc.sync.dma_start(out=gb, in_=bcast_ap(gate[n:n + 1, :]))
            cb = con.tile([128, D], F32, tag=f"cost32b{n}")
            nc.vector.tensor_scalar(out=cb, in0=gb, scalar1=invb[:, n:n + 1],
                                    scalar2=32.0, op0=ALU.mult, op1=ALU.mult)
            cost32b[n] = cb
            mb = con.tile([128, D], F32, tag=f"mub_b{n}")
            nc.sync.dma_start(out=mb, in_=bcast_ap(mu_b[n:n + 1, :]))
            mub_b[n] = mb

        def layernorm_rows(pool, dst, src_t, gt, bt):
            sub = src_t.rearrange("p (a q) -> p a q", a=2)
            stats = pool.tile([128, 2, 6], F32, tag="lnst")
            for a in range(2):
                nc.vector.bn_stats(out=stats[:, a, :], in_=sub[:, a, :])
            mv = pool.tile([128, 2], F32, tag="lnmv")
            nc.vector.bn_aggr(out=mv, in_=stats)
            rstd = pool.tile([128, 1], F32, tag="lnr")
            nc.scalar.activation(out=rstd, in_=mv[:, 1:2], func=ACTF.Sqrt,
                                 bias=eps_t, scale=1.0)
            nc.vector.reciprocal(rstd, rstd)
            nc.vector.tensor_scalar(out=dst, in0=src_t, scalar1=mv[:, 0:1],
                                    scalar2=rstd, op0=ALU.subtract, op1=ALU.mult)
            nc.vector.tensor_tensor(out=dst, in0=dst, in1=gt, op=ALU.mult)
            nc.vector.tensor_tensor(out=dst, in0=dst, in1=bt, op=ALU.add)

        def transpose128(dst, src):
            pt = pst.tile([128, 128], F32, tag="tp")
            nc.tensor.transpose(pt, src, ident)
            nc.scalar.copy(dst, pt)

        # ---------- proto_state slices + AllGather (per n) ----------
        with tc.tile_pool(name="proto", bufs=1) as pp:
            for n in range(4):
                plg = pp.tile([128, D], F32, tag="plg")
                nc.sync.dma_start(out=plg, in_=bcast_ap(pln_g[n:n + 1, :]))
                plb = pp.tile([128, D], F32, tag="plb")
                nc.sync.dma_start(out=plb, in_=bcast_ap(pln_b[n:n + 1, :]))
                ipT = pp.tile([128, NT, 128], F32, tag="ipT")
                nc.sync.dma_start(
                    out=ipT, in_=ip_Ts[n].rearrange("(i p) u -> p i u", p=128))
                ptT = pp.tile([128, NT, D], F32, tag="ptT")
                nc.sync.dma_start(
                    out=ptT, in_=pt_wT[n].rearrange("(i p) w -> p i w", p=128))
                prc = pp.tile([128, D], F32, tag="prc")
                for c in range(2):
                    pm = ps.tile([128, 512], F32, tag="mm")
                    for i in range(NT):
                        nc.tensor.matmul(pm, ipT[:, i, :],
                                         ptT[:, i, c * 512:(c + 1) * 512],
                                         start=(i == 0), stop=(i == NT - 1))
                    nc.scalar.copy(prc[:, c * 512:(c + 1) * 512], pm)
                psl = pp.tile([128, D], F32, tag="psl")
                layernorm_rows(pp, psl, prc, plg, plb)
                pw = pp.tile([128, D], F32, tag="pw")
                nc.sync.dma_start(out=pw, in_=pw_s[n])
                nc.vector.tensor_tensor(out=psl, in0=psl, in1=pw, op=ALU.add)
                psT = pp.tile([128, NT, 128], F32, tag="psT")
                for i in range(NT):
                    transpose128(psT[:, i, :], psl[:, i * 128:(i + 1) * 128])
                nc.sync.dma_start(
                    out=proto_in[n][:, :].rearrange("(i p) u -> p i u", p=128),
                    in_=psT)
                if use_cc:
                    nc.gpsimd.collective_compute(
                        "AllGather", ALU.bypass, replica_groups=[list(range(8))],
                        ins=[proto_in[n][:, :].opt()],
                        outs=[proto_all[n][:, :].opt()])
                else:
                    nc.sync.dma_start(out=proto_all[n][0:D, :],
                                      in_=proto_in[n][:, :])

        # ---------- h = LN(x) -> hT; q,k,v; kv exchange ----------
        hT_stack = ExitStack()
        pool_hT = hT_stack.enter_context(tc.tile_pool(name="hTp", bufs=1))
        hT = [pool_hT.tile([128, SLOC], F32, tag=f"hT{i}", name=f"hT{i}") for i in range(NT)]
        with tc.tile_pool(name="hwrk", bufs=2) as hw:
            gb_ = hw.tile([128, D], F32, tag="g")
            nc.sync.dma_start(out=gb_, in_=bcast_ap(ln1_g[0:1, :]))
            bb_ = hw.tile([128, D], F32, tag="b")
            nc.sync.dma_start(out=bb_, in_=bcast_ap(ln1_b[0:1, :]))
            for st in range(ST):
                xt = hw.tile([128, D], F32, tag="xt")
                nc.sync.dma_start(out=xt, in_=x_own[st * 128:(st + 1) * 128, :])
                ht = hw.tile([128, D], F32, tag="ht")
                layernorm_rows(hw, ht, xt, gb_, bb_)
                for i in range(NT):
                    transpose128(hT[i][:, st * 128:(st + 1) * 128],
                                 ht[:, i * 128:(i + 1) * 128])

        qT_stack = ExitStack()
        pool_qT = qT_stack.enter_context(tc.tile_pool(name="qTp", bufs=1, side="right"))
        qT = [pool_qT.tile([128, SLOC], F32, tag=f"qT{i}", name=f"qT{i}") for i in range(NT)]

        # q, k in T-layout
        with tc.tile_pool(name="statqk", bufs=2) as sq, \
             tc.tile_pool(name="ktp", bufs=2) as kp:
            for n in range(2):
                for j in range(NT):
                    mus = sq.tile([128, NT, 128], F32, tag="mus")
                    nc.sync.dma_start(
                        out=mus,
                        in_=mu_wT[n, :, j * 128:(j + 1) * 128].rearrange(
                            "(i p) u -> p i u", p=128))
                    prs = sq.tile([128, NT, 128], F32, tag="prs")
                    for i in range(NT):
                        nc.sync.dma_start(out=prs[:, i, :], in_=protoT_tile(j, n, i))
                    dst = (qT[j] if n == 0 else
                           kp.tile([128, SLOC], F32, tag="kT", name="kTj"))
                    for c in range(2):
                        sl = slice(c * 512, (c + 1) * 512)
                        pc = ps.tile([128, 512], F32, tag="mm")
                        for i in range(NT):
                            nc.tensor.matmul(pc, mus[:, i, :], hT[i][:, sl],
                                             start=(i == 0), stop=(i == NT - 1))
                        pm = ps.tile([128, 512], F32, tag="mm2")
                        for i in range(NT):
                            nc.tensor.matmul(pm, prs[:, i, :], hT[i][:, sl],
                                             start=(i == 0), stop=(i == NT - 1))
                        comp = wrk.tile([128, 512], F32, tag="comp")
                        nc.scalar.activation(out=comp, in_=pc, func=ACTF.Silu,
                                             bias=mub_t[:, n, j:j + 1], scale=1.0)
                        msk = wrk.tile([128, 512], F32, tag="msk")
                        nc.vector.tensor_scalar(
                            out=msk, in0=pm, scalar1=cost32_t[:, n, j:j + 1],
                            scalar2=0.0, op0=ALU.subtract, op1=ALU.is_gt)
                        nc.vector.tensor_tensor(out=dst[:, sl], in0=comp, in1=msk,
                                                op=ALU.mult)
                    if n == 1:
                        nc.sync.dma_start(
                            out=kv_in[j * 128:(j + 1) * 128, :], in_=dst)

        # v in N-layout, written straight to kv_in
        with tc.tile_pool(name="statv", bufs=1) as sv:
            for uc in range(4):
                usl = slice(uc * 256, (uc + 1) * 256)
                muv = sv.tile([128, NT, 256], F32, tag="muv")
                nc.sync.dma_start(
                    out=muv,
                    in_=mu_wT[2, :, usl].rearrange("(i p) u -> p i u", p=128))
                prv = sv.tile([128, NT, 256], F32, tag="prv")
                for i in range(NT):
                    for jj in range(2):
                        j = uc * 2 + jj
                        nc.sync.dma_start(out=prv[:, i, jj * 128:(jj + 1) * 128],
                                          in_=protoT_tile(j, 2, i))
                for st in range(ST):
                    ssl = slice(st * 128, (st + 1) * 128)
                    pc = ps.tile([128, 512], F32, tag="mm")
                    for i in range(NT):
                        nc.tensor.matmul(pc[:, 0:256], hT[i][:, ssl], muv[:, i, :],
                                         start=(i == 0), stop=(i == NT - 1))
                    pm = ps.tile([128, 512], F32, tag="mm2")
                    for i in range(NT):
                        nc.tensor.matmul(pm[:, 0:256], hT[i][:, ssl], prv[:, i, :],
                                         start=(i == 0), stop=(i == NT - 1))
                    comp = wrk.tile([128, 512], F32, tag="comp")
                    nc.vector.tensor_tensor(out=comp[:, 0:256], in0=pc[:, 0:256],
                                            in1=mub_b[2][:, usl], op=ALU.add)
                    nc.scalar.activation(out=comp[:, 0:256], in_=comp[:, 0:256],
                                         func=ACTF.Silu)
                    msk = wrk.tile([128, 512], F32, tag="msk")
                    nc.vector.tensor_tensor(out=msk[:, 0:256], in0=pm[:, 0:256],
                                            in1=cost32b[2][:, usl], op=ALU.subtract)
                    nc.vector.tensor_scalar(out=msk[:, 0:256], in0=msk[:, 0:256],
                                            scalar1=0.0, scalar2=None, op0=ALU.is_gt)
                    vout = wrk.tile([128, 256], F32, tag="vout")
                    nc.vector.tensor_tensor(out=vout, in0=comp[:, 0:256],
                                            in1=msk[:, 0:256], op=ALU.mult)
                    nc.sync.dma_start(
                        out=kv_in[SLOC + st * 128:SLOC + (st + 1) * 128, usl],
                        in_=vout)
        hT_stack.close()

        if use_cc:
            nc.gpsimd.collective_compute(
                "AllGather", ALU.bypass,
                replica_groups=[[0, 1], [2, 3], [4, 5], [6, 7]],
                ins=[kv_in[:, :].opt()], outs=[kv_all[:, :].opt()])
        else:
            nc.sync.dma_start(out=kv_all[0:2 * SLOC, :], in_=kv_in[:, :])
            nc.sync.dma_start(out=kv_all[2 * SLOC:4 * SLOC, :], in_=kv_in[:, :])

        # ---------- scores + softmax ----------
        P_stack = ExitStack()
        pool_P = P_stack.enter_context(tc.tile_pool(name="Pp", bufs=1))
        P = [pool_P.tile([128, LT[t]], F32, tag=f"P{t}", name=f"P{t}") for t in range(ST)]
        with tc.tile_pool(name="statk", bufs=2) as sk:
            for c in range(MAXCHUNK):
                kch = sk.tile([128, NT, 512], F32, tag="kch")
                for i in range(NT):
                    for kb in range(4):
                        tau = c * 4 + kb
                        p_, l_ = KEYSRC[tau]
                        r0 = p_ * 2 * SLOC + i * 128
                        nc.sync.dma_start(
                            out=kch[:, i, kb * 128:(kb + 1) * 128],
                            in_=kv_all[r0:r0 + 128, l_ * 128:(l_ + 1) * 128])
                for t in range(ST):
                    if LT[t] <= c * 512:
                        continue
                    n_c = min(512, LT[t] - c * 512)
                    pp_ = ps.tile([128, 512], F32, tag="mm")
                    for i in range(NT):
                        nc.tensor.matmul(pp_[:, :n_c],
                                         qT[i][:, t * 128:(t + 1) * 128],
                                         kch[:, i, :n_c],
                                         start=(i == 0), stop=(i == NT - 1))
                    mt = wrk.tile([128, 512], F32, tag="amsk")
                    nc.sync.dma_start(out=mt[:, :n_c],
                                      in_=amask[t, :, c * 512:c * 512 + n_c])
                    seg = P[t][:, c * 512:c * 512 + n_c]
                    nc.scalar.activation(out=seg, in_=pp_[:, :n_c], func=ACTF.Copy,
                                         scale=SCALE)
                    nc.vector.tensor_tensor(out=seg, in0=seg, in1=mt[:, :n_c],
                                            op=ALU.add)
        qT_stack.close()

        at_stack = ExitStack()
        pool_at = at_stack.enter_context(tc.tile_pool(name="atp", bufs=1, side="right"))
        rinv = pool_at.tile([128, ST], F32, tag="rinv")
        for t in range(ST):
            nmax = wrk.tile([128, 1], F32, tag="nmax")
            nc.vector.tensor_reduce(nmax, P[t], AX.X, ALU.max, negate=True)
            nc.scalar.activation(out=P[t], in_=P[t], func=ACTF.Exp, bias=nmax,
                                 scale=1.0)
            rs = wrk.tile([128, 1], F32, tag="rs")
            nc.vector.tensor_reduce(rs, P[t], AX.X, ALU.add)
            nc.vector.reciprocal(rinv[:, t:t + 1], rs)

        # ---------- attn = P @ v ----------
        attn = [pool_at.tile([128, D], F32, tag=f"at{t}", name=f"at{t}") for t in range(ST)]
        with tc.tile_pool(name="statvch", bufs=2) as sva:
            for c in range(MAXCHUNK):
                vch = sva.tile([128, 4, D], F32, tag="vch")
                for kb in range(4):
                    tau = c * 4 + kb
                    p_, l_ = KEYSRC[tau]
                    r0 = p_ * 2 * SLOC + SLOC + l_ * 128
                    nc.sync.dma_start(out=vch[:, kb, :], in_=kv_all[r0:r0 + 128, :])
                for t in range(ST):
                    if LT[t] <= c * 512:
                        continue
                    n_c = min(512, LT[t] - c * 512)
                    nkb = n_c // 128
                    PT = wrk.tile([128, 4, 128], F32, tag="PT")
                    for kb in range(nkb):
                        transpose128(
                            PT[:, kb, :],
                            P[t][:, c * 512 + kb * 128:c * 512 + (kb + 1) * 128])
                    for fc in range(2):
                        fsl = slice(fc * 512, (fc + 1) * 512)
                        pa = ps.tile([128, 512], F32, tag="mm")
                        for kb in range(nkb):
                            nc.tensor.matmul(pa, PT[:, kb, :], vch[:, kb, fsl],
                                             start=(kb == 0), stop=(kb == nkb - 1))
                        if c == 0:
                            nc.scalar.copy(attn[t][:, fsl], pa)
                        else:
                            nc.vector.tensor_tensor(out=attn[t][:, fsl],
                                                    in0=attn[t][:, fsl], in1=pa,
                                                    op=ALU.add)
        P_stack.close()
        for t in range(ST):
            nc.vector.tensor_scalar_mul(attn[t], attn[t], rinv[:, t:t + 1])

        # ---------- attn_outT; out-proj + residual ----------
        aT_stack = ExitStack()
        pool_aT = aT_stack.enter_context(tc.tile_pool(name="aTp", bufs=1))
        aT = [pool_aT.tile([128, SLOC], F32, tag=f"aT{i}", name=f"aT{i}") for i in range(NT)]
        for t in range(ST):
            for i in range(NT):
                transpose128(aT[i][:, t * 128:(t + 1) * 128],
                             attn[t][:, i * 128:(i + 1) * 128])
        at_stack.close()

        with tc.tile_pool(name="stato", bufs=1) as so:
            for uc in range(4):
                usl = slice(uc * 256, (uc + 1) * 256)
                muo = so.tile([128, NT, 256], F32, tag="muo")
                nc.sync.dma_start(
                    out=muo,
                    in_=mu_wT[3, :, usl].rearrange("(i p) u -> p i u", p=128))
                pro = so.tile([128, NT, 256], F32, tag="pro")
                for i in range(NT):
                    for jj in range(2):
                        j = uc * 2 + jj
                        nc.sync.dma_start(out=pro[:, i, jj * 128:(jj + 1) * 128],
                                          in_=protoT_tile(j, 3, i))
                for st in range(ST):
                    ssl = slice(st * 128, (st + 1) * 128)
                    pc = ps.tile([128, 512], F32, tag="mm")
                    for i in range(NT):
                        nc.tensor.matmul(pc[:, 0:256], aT[i][:, ssl], muo[:, i, :],
                                         start=(i == 0), stop=(i == NT - 1))
                    pm = ps.tile([128, 512], F32, tag="mm2")
                    for i in range(NT):
                        nc.tensor.matmul(pm[:, 0:256], aT[i][:, ssl], pro[:, i, :],
                                         start=(i == 0), stop=(i == NT - 1))
                    comp = wrk.tile([128, 512], F32, tag="comp")
                    nc.vector.tensor_tensor(out=comp[:, 0:256], in0=pc[:, 0:256],
                                            in1=mub_b[3][:, usl], op=ALU.add)
                    nc.scalar.activation(out=comp[:, 0:256], in_=comp[:, 0:256],
                                         func=ACTF.Silu)
                    msk = wrk.tile([128, 512], F32, tag="msk")
                    nc.vector.tensor_tensor(out=msk[:, 0:256], in0=pm[:, 0:256],
                                            in1=cost32b[3][:, usl], op=ALU.subtract)
                    nc.vector.tensor_scalar(out=msk[:, 0:256], in0=msk[:, 0:256],
                                            scalar1=0.0, scalar2=None, op0=ALU.is_gt)
                    nc.vector.tensor_tensor(out=msk[:, 0:256], in0=comp[:, 0:256],
                                            in1=msk[:, 0:256], op=ALU.mult)
                    xt = wrk.tile([128, 256], F32, tag="xres")
                    nc.sync.dma_start(out=xt, in_=x_own[ssl, usl])
                    nc.vector.tensor_tensor(out=msk[:, 0:256], in0=msk[:, 0:256],
                                            in1=xt, op=ALU.add)
                    nc.sync.dma_start(out=y[ssl, usl], in_=msk[:, 0:256])
        aT_stack.close()

    nc.compile()
    return nc


# ======================= host side =======================

_RUNNER = None


class _SpmdRunner:
    def __init__(self, nc, n_cores):
        import jax
        from jax.sharding import Mesh, PartitionSpec
        from jax.experimental.shard_map import shard_map
        from concourse.bass2jax import (
            _bass_exec_p, partition_id_tensor, install_neuronx_cc_hook)

        install_neuronx_cc_hook()
        self.jax = jax
        self.n_cores = n_cores
        partition_name = nc.partition_id_tensor.name if nc.partition_id_tensor else None
        in_names, out_names, out_avals = [], [], []
        for alloc in nc.m.functions[0].allocations:
            if not isinstance(alloc, mybir.MemoryLocationSet):
                continue
            name = alloc.memorylocations[0].name
            if alloc.kind == "ExternalInput":
                if name != partition_name:
                    in_names.append(name)
            elif alloc.kind == "ExternalOutput":
                out_names.append(name)
                out_avals.append(jax.core.ShapedArray(
                    tuple(alloc.tensor_shape), mybir.dt.np(alloc.dtype)))
        self.in_names, self.out_names, self.out_avals = in_names, out_names, out_avals
        n_params, n_outs = len(in_names), len(out_avals)

        def _body(*args):
            operands = list(args)
            if partition_name is not None:
                operands.append(partition_id_tensor())
            outs = _bass_exec_p.bind(
                *operands,
                out_avals=tuple(out_avals),
                in_names=tuple(in_names + out_names
                               + ([partition_name] if partition_name else [])),
                out_names=tuple(out_names),
                lowering_input_output_aliases=(),
                sim_require_finite=False,
                sim_require_nnan=False,
                nc=nc,
            )
            return tuple(outs)

        devices = jax.devices()[:n_cores]
        mesh = Mesh(np.asarray(devices), ("core",))
        in_specs = (PartitionSpec("core"),) * (n_params + n_outs)
        out_specs = (PartitionSpec("core"),) * n_outs
        self.fn = jax.jit(
            shard_map(_body, mesh=mesh, in_specs=in_specs, out_specs=out_specs,
                      check_rep=False),
            keep_unused=True)

    def run(self, in_maps):
        concat_in = [
            np.ascontiguousarray(
                np.concatenate([np.asarray(m[name]) for m in in_maps], axis=0))
            for name in self.in_names
        ]
        concat_zeros = [
            np.zeros((self.n_cores * a.shape[0], *a.shape[1:]), a.dtype)
            for a in self.out_avals
        ]
        outs = self.fn(*concat_in, *concat_zeros)
        self.jax.block_until_ready(outs)
        res = []
        for c in range(self.n_cores):
            d = {}
            for i, name in enumerate(self.out_names):
                full = np.asarray(outs[i])
                d[name] = full.reshape(self.n_cores, *self.out_avals[i].shape)[c]
            res.append(d)
        return res


def _build_mask(h):
    m = np.full((ST, 128, S), NEG, np.float32)
    order = TILE_ORDER[h]
    keys = np.arange(S)
    for t, tau in enumerate(order):
        rows = tau * 128 + np.arange(128)
        m[t] = np.where(keys[None, :] <= rows[:, None], 0.0, NEG)
        m[t, :, LT[t]:] = NEG
    return m


def make_in_maps(x, ln1_g, ln1_b, mu_w, mu_b, proto_w, gate, pt_w, pln_g,
                 pln_b, in_proto):
    mu_wT = np.ascontiguousarray(np.transpose(mu_w, (0, 2, 1)))
    pt_wT = np.ascontiguousarray(np.transpose(pt_w, (0, 2, 1)))
    ident = np.eye(128, dtype=np.float32)
    masks = {h: _build_mask(h) for h in (0, 1)}
    in_maps = []
    for c in range(8):
        b, h = c // 2, c % 2
        order = TILE_ORDER[h]
        x_own = np.concatenate(
            [x[b, tau * 128:(tau + 1) * 128, :] for tau in order], axis=0)
        ip_Ts = np.ascontiguousarray(
            np.transpose(in_proto[:, c * 128:(c + 1) * 128, :], (0, 2, 1)))
        pw_s = np.ascontiguousarray(proto_w[:, c * 128:(c + 1) * 128, :])
        in_maps.append({
            "x_own": np.ascontiguousarray(x_own),
            "ln1_g": np.asarray(ln1_g).reshape(1, D),
            "ln1_b": np.asarray(ln1_b).reshape(1, D),
            "mu_wT": mu_wT, "mu_b": mu_b, "gate": gate,
            "pt_wT": pt_wT, "pln_g": pln_g, "pln_b": pln_b,
            "ip_Ts": ip_Ts, "pw_s": pw_s,
            "amask": masks[h], "ident_in": ident,
        })
    return in_maps


def kernel(**inputs):
    global _RUNNER
    inputs = {k: np.asarray(v, np.float32) for k, v in inputs.items()}
    in_maps = make_in_maps(**inputs)
    if _RUNNER is None:
        _RUNNER = _SpmdRunner(build(), 8)
    res = _RUNNER.run(in_maps)
    out = np.empty((B, S, D), np.float32)
    for c in range(8):
        b, h = c // 2, c % 2
        yc = res[c]["y"]
        for t, tau in enumerate(TILE_ORDER[h]):
            out[b, tau * 128:(tau + 1) * 128, :] = yc[t * 128:(t + 1) * 128, :]
    return out
